# revision 1
# baseline (speedup 1.0000x reference)
"""Trainium2 Bass kernel for nn_Node3DEmbeddingv2 (gnn_message_passing).

Strategy (8 NeuronCores, SPMD, data-parallel over flattened (batch, query-row)):
  - 1536 query rows split into 8 x 192 (batch-aligned: 4 cores per batch).
  - Per core, per 32-row group: pairwise deltas vs all 768 keys on DVE,
    squared, reduced over xyz via a block-replicating matmul -> d^2 (PSUM),
    ACT sqrt -> d (replicated 3x across partitions).
  - d is split into 3 bf16 components (hi/mid/lo, exact to f32 precision);
    a [3,128]-ones bf16 matmul broadcasts each row's 768 distances across
    all 128 gaussian-channel partitions at full PE rate (f32 accumulate).
  - One ScalarE op per row computes the whole Gaussian:
      Derivative_Erf(scale_k * d + bias_k) = 2/sqrt(pi) * exp(-((d-m_k)/s_k)^2/2)
    with accum_out producing the sum over the 768 keys for free.
    (Fallback path: Square + Exp, two ACT passes, if the table is inaccurate.)
  - Channel constants 1/(sqrt(2 pi) s_k) (and the table constant) are applied
    post-reduction on the tiny [128, 192] summed tensor.
  - feature_proj MLP (gelu between two matmuls) on device; PE transposes the
    [E, rows] result back to row-major, adds the host-computed angle/time
    embedding tail, DMAs out [192, 512] per core.
  - Host (numpy, negligible): angle MLP, sinusoidal time embedding MLP,
    masking, per-core input prep; all heavy compute is on-device.
"""

import math

import numpy as np

# Problem constants (hardcoded per the task contract).
B, N, K, E = 2, 768, 128, 512
INTER = E // 2
NCORES = 8
RPC = (B * N) // NCORES  # 192 rows per core
GR = 32                  # rows per group
NGROUPS = RPC // GR      # 6 groups per core
PI_REF = 3.14159         # matches reference's gaussian constant

# Derivative_Erf table semantics: d/dx erf(x) = 2/sqrt(pi) * exp(-x^2).
# DERF_INV is the factor that converts the table output back to exp(-x^2).
DERF_INV = math.sqrt(math.pi) / 2.0

# Set to True to use the Square+Exp fallback instead of Derivative_Erf.
USE_FALLBACK_EXP = False
# Simulator/testing hook: replace Derivative_Erf by another func (e.g. Square).
_FUNC_OVERRIDE = None

_COMPILED = {}


def _enable_ldw_opt():
    """Flip walrus's redundant-LDWEIGHTS elimination on: our 384 broadcast
    matmuls reuse one stationary [3,128] ones matrix, and the per-matmul
    reload serializes ~134ns each on the PE. Correctness is re-verified
    end-to-end against the oracle after any compile-flag change."""
    from concourse import bass_utils

    if getattr(bass_utils, "_ldw_opt_patched", False):
        return
    orig = bass_utils.bir_verify_and_optimise

    def patched(*args, **kwargs):
        import subprocess

        orig_run = bass_utils.run_command

        def run_patched(argv, **kw):
            argv = [
                a.replace("--enable-ldw-opt=false", "--enable-ldw-opt=true")
                if isinstance(a, str) else a
                for a in argv
            ]
            return orig_run(argv, **kw)

        bass_utils.run_command = run_patched
        try:
            return orig(*args, **kwargs)
        finally:
            bass_utils.run_command = orig_run

    bass_utils.bir_verify_and_optimise = patched
    bass_utils._ldw_opt_patched = True


def _build_nc(use_fallback=None, func_override=None, gelu_override=None):
    import concourse.bass as bass
    import concourse.bacc as bacc
    from concourse import mybir
    from concourse.tile import TileContext

    # note: _enable_ldw_opt() breaks walrus codegen (standalone InstLdweights
    # with f32 matmuls in the module) — left available but unused

    if use_fallback is None:
        use_fallback = USE_FALLBACK_EXP
    f32 = mybir.dt.float32
    bf16 = mybir.dt.bfloat16
    AF = mybir.ActivationFunctionType

    nc = bacc.Bacc("TRN2", target_bir_lowering=False)

    # DRAM I/O (per-core values supplied via in_maps).
    posT = nc.dram_tensor("posT", [3, N], f32, kind="ExternalInput")
    qscal = nc.dram_tensor("qscal", [96, NGROUPS], f32, kind="ExternalInput")
    blk3 = nc.dram_tensor("blk3", [96, GR], f32, kind="ExternalInput")
    esc = nc.dram_tensor("esc", [K, 1], f32, kind="ExternalInput")
    ebi = nc.dram_tensor("ebi", [K, 1], f32, kind="ExternalInput")
    postc = nc.dram_tensor("postc", [K, 1], f32, kind="ExternalInput")
    w1 = nc.dram_tensor("w1", [K, K], f32, kind="ExternalInput")
    w2 = nc.dram_tensor("w2", [K, INTER], f32, kind="ExternalInput")
    ident = nc.dram_tensor("ident", [128, 128], f32, kind="ExternalInput")
    rest = nc.dram_tensor("rest", [RPC, E], f32, kind="ExternalInput")
    out = nc.dram_tensor("out", [RPC, E], f32, kind="ExternalOutput")

    with TileContext(nc) as tc:
        with tc.tile_pool(name="sb", bufs=1) as sb:
            # ---- constant loads ----
            pos_rep = sb.tile([96, N], f32, tag="pos_rep")
            nc.sync.dma_start(
                out=pos_rep,
                in_=bass.AP(tensor=posT, offset=0, ap=[[0, 32], [N, 3], [1, N]]),
            )
            q_sb = sb.tile([96, NGROUPS], f32, tag="q_sb")
            nc.sync.dma_start(out=q_sb, in_=qscal[:, :])
            blk_sb = sb.tile([96, GR], f32, tag="blk_sb")
            nc.sync.dma_start(out=blk_sb, in_=blk3[:, :])
            esc_sb = sb.tile([K, 1], f32, tag="esc_sb")
            nc.sync.dma_start(out=esc_sb, in_=esc[:, :])
            ebi_sb = sb.tile([K, 1], f32, tag="ebi_sb")
            nc.sync.dma_start(out=ebi_sb, in_=ebi[:, :])
            postc_sb = sb.tile([K, 1], f32, tag="postc_sb")
            nc.sync.dma_start(out=postc_sb, in_=postc[:, :])
            w1_sb = sb.tile([K, K], f32, tag="w1_sb")
            nc.sync.dma_start(out=w1_sb, in_=w1[:, :])
            w2_sb = sb.tile([K, INTER], f32, tag="w2_sb")
            nc.sync.dma_start(out=w2_sb, in_=w2[:, :])
            id_sb = sb.tile([128, 128], f32, tag="id_sb")
            nc.sync.dma_start(out=id_sb, in_=ident[:, :])
            ones3 = sb.tile([3, 128], bf16, tag="ones3")
            nc.vector.memset(ones3, 1.0)

            S = sb.tile([K, RPC], f32, tag="S")

            # Collapse the many input-DMA queue semaphores into one point so
            # downstream consumers never need more waits than the instruction
            # encoding allows.
            tc.strict_bb_all_engine_barrier()

            # ---- phase A: distances + bf16 splits for all 6 groups ----
            split_tiles = []
            with tc.tile_pool(name="psA", bufs=1, space="PSUM") as psA:
                for g in range(NGROUPS):
                    delta = sb.tile([96, N], f32, tag="delta", bufs=2)
                    nc.vector.tensor_scalar(
                        out=delta,
                        in0=pos_rep,
                        scalar1=q_sb[:, g : g + 1],
                        scalar2=None,
                        op0=mybir.AluOpType.subtract,
                    )
                    nc.vector.tensor_mul(delta, delta, delta)
                    psum_d2 = psA.tile([GR, N], f32, tag="d2", bufs=2)
                    nc.tensor.matmul(
                        psum_d2[:, 0:512], blk_sb, delta[:, 0:512],
                        start=True, stop=True,
                    )
                    nc.tensor.matmul(
                        psum_d2[:, 512:N], blk_sb, delta[:, 512:N],
                        start=True, stop=True,
                    )
                    d_sb = sb.tile([GR, N], f32, tag=f"d{g}")
                    nc.scalar.sqrt(d_sb, psum_d2)
                    # exact 3-way bf16 split: hi + mid + lo == d (f32 precision)
                    dh = sb.tile([GR, N], bf16, tag=f"dh{g}")
                    nc.vector.tensor_copy(dh, d_sb)
                    r1 = sb.tile([GR, N], f32, tag="r1", bufs=2)
                    nc.vector.tensor_sub(r1, d_sb, dh)
                    dm = sb.tile([GR, N], bf16, tag=f"dm{g}")
                    nc.vector.tensor_copy(dm, r1)
                    r2 = sb.tile([GR, N], f32, tag="r2", bufs=2)
                    nc.vector.tensor_sub(r2, r1, dm)
                    dl = sb.tile([GR, N], bf16, tag=f"dl{g}")
                    nc.vector.tensor_copy(dl, r2)
                    split_tiles.append((dh, dm, dl))

            # ---- phase B: broadcast + gaussian + key-sum per row ----
            derf_func = AF.Derivative_Erf
            if func_override is not None:
                derf_func = func_override
            with tc.tile_pool(name="psB", bufs=1, space="PSUM") as psB:
                ones2q = sb.tile([67, 128], bf16, tag="ones2q")
                nc.vector.memset(ones2q, 1.0)
                SG = GR // 2
                for g2 in range(NGROUPS * 2):
                    g, s = divmod(g2, 2)
                    # flatten 16 rows onto two PE quadrant trios (partitions
                    # 0..2 and 64..66): 8 rows each. Alternating matmul issue
                    # between the quadrants lets the PE pipeline them
                    # concurrently (~2x effective rate).
                    M_flat = sb.tile([67, SG * N // 2], bf16, tag="mflat", bufs=2)
                    for c, comp in enumerate(split_tiles[g]):
                        for q in range(2):
                            nc.gpsimd.dma_start(
                                out=M_flat[
                                    64 * q + c : 64 * q + c + 1, :
                                ].rearrange("p (a j) -> p a j", a=SG // 2),
                                in_=comp[
                                    SG * s + 8 * q : SG * s + 8 * (q + 1), :
                                ],
                            )
                    for a0 in range(0, SG, 4):
                        a = SG * s + a0
                        r = g * GR + a
                        unit = r // 4
                        # every 8th 4-row unit sums on the ScalarE accumulator
                        # (1-row activations with accum_out) to offload the DVE
                        act_accum_unit = (not use_fallback) and unit % 8 == 7
                        # 4-row macro unit: two 2-row PSUM tiles -> one 4-row
                        # gsc tile. Rows a0, a0+1 come from quadrant 0
                        # (partitions 0..2), rows a0+2, a0+3 (= slot a0, a0+1
                        # of the upper half) from quadrant 2 (partitions
                        # 64..66); issue alternates between the two so their
                        # matmuls overlap in the PE array.
                        gsc = sb.tile([K, 4, N], f32, tag="gsc", bufs=4)
                        mms = []
                        psums = []
                        base = (a0 // 4) * 2 * N  # slot pair 2u, 2u+1
                        for h in range(2):
                            psum_db = psB.tile([K, 2, N], f32, tag="db", bufs=2)
                            flat = psum_db.rearrange("k a j -> k (a j)")
                            qb = 64 * h
                            for lo in (0, 512, 1024):
                                mms.append(
                                    (
                                        flat[:, lo : lo + 512],
                                        ones2q[qb : qb + 3, :],
                                        M_flat[qb : qb + 3, base + lo : base + lo + 512],
                                        (qb, 0),
                                    )
                                )
                            psums.append(psum_db)
                        for idx in (0, 3, 1, 4, 2, 5):
                            out_ap, lhsT, rhs, tp = mms[idx]
                            nc.tensor.matmul(
                                out_ap, lhsT, rhs,
                                start=True, stop=True, tile_position=tp,
                            )
                        for h in range(2):
                            psum_db = psums[h]
                            if act_accum_unit:
                                for q in range(2):
                                    nc.scalar.activation(
                                        out=gsc[:, 2 * h + q, :],
                                        in_=psum_db[:, q, :],
                                        func=derf_func,
                                        bias=ebi_sb,
                                        scale=esc_sb,
                                        accum_out=S[:, r + 2 * h + q : r + 2 * h + q + 1],
                                    )
                            elif not use_fallback:
                                nc.scalar.activation(
                                    out=gsc[:, 2 * h : 2 * h + 2, :],
                                    in_=psum_db,
                                    func=derf_func,
                                    bias=ebi_sb,
                                    scale=esc_sb,
                                )
                            else:
                                zsq = sb.tile([K, 2, N], f32, tag="zsq", bufs=3)
                                nc.scalar.activation(
                                    out=zsq, in_=psum_db,
                                    func=AF.Square, bias=ebi_sb, scale=esc_sb,
                                )
                                nc.scalar.activation(
                                    out=gsc[:, 2 * h : 2 * h + 2, :], in_=zsq,
                                    func=AF.Exp, bias=postc_sb, scale=-0.5,
                                )
                        if not act_accum_unit:
                            # key-axis sum on DVE (4 rows per op)
                            nc.vector.reduce_sum(
                                out=S[:, r : r + 4], in_=gsc,
                                axis=mybir.AxisListType.X,
                            )

            # ---- phase C: channel constants + feature_proj MLP + output ----
            # processed in two 96-row chunks so the second half of phase B can
            # still be running while the first chunk's MLP drains
            with tc.tile_pool(name="psC", bufs=1, space="PSUM") as psC:
                gelu_func = AF.Gelu if gelu_override is None else gelu_override
                for t in range(2):
                    rows = slice(96 * t, 96 * (t + 1))
                    if not use_fallback:
                        nc.vector.tensor_scalar_mul(
                            S[:, rows], S[:, rows], postc_sb
                        )
                    psum_h = psC.tile([K, 96], f32, tag="mlp", bufs=2)
                    nc.tensor.matmul(psum_h, w1_sb, S[:, rows], start=True, stop=True)
                    h_sb = sb.tile([K, 96], f32, tag="h_sb", bufs=2)
                    nc.scalar.activation(h_sb, psum_h, gelu_func)
                    o_sb = sb.tile([128, 2, 96], f32, tag="o_sb", bufs=2)
                    for e in range(2):
                        psum_o = psC.tile([128, 96], f32, tag="mlp", bufs=2)
                        nc.tensor.matmul(
                            psum_o, w2_sb[:, 128 * e : 128 * (e + 1)], h_sb,
                            start=True, stop=True,
                        )
                        nc.vector.tensor_copy(o_sb[:, e, :], psum_o)
                    out_sb = sb.tile([96, E], f32, tag=f"out{t}")
                    nc.gpsimd.dma_start(
                        out=out_sb, in_=rest[96 * t : 96 * (t + 1), :]
                    )
                    for e in range(2):
                        psum_t = psC.tile([96, 128], f32, tag="tr", bufs=2)
                        nc.tensor.transpose(psum_t, o_sb[:, e, :], id_sb)
                        nc.vector.tensor_add(
                            out_sb[:, 128 * e : 128 * (e + 1)],
                            out_sb[:, 128 * e : 128 * (e + 1)],
                            psum_t,
                        )
                    nc.sync.dma_start(
                        out=out[96 * t : 96 * (t + 1), :], in_=out_sb
                    )

    nc.compile()
    return nc


# ---------------- host-side reference tails (numpy, f32) ----------------

def _erf_np(x):
    try:
        from scipy.special import erf
        return erf(x).astype(np.float32)
    except ImportError:
        f = np.frompyfunc(math.erf, 1, 1)
        return f(x.astype(np.float64)).astype(np.float32)


def _gelu_np(x):
    x = x.astype(np.float32)
    return (x * 0.5 * (1.0 + _erf_np(x / np.float32(math.sqrt(2.0))))).astype(
        np.float32
    )


def _silu_np(x):
    x = x.astype(np.float32)
    return (x / (1.0 + np.exp(-x))).astype(np.float32)


def _timestep_emb_np(t, dim):
    half = dim // 2
    freqs = np.exp(
        -np.log(10000.0) * np.arange(half, dtype=np.float32) / np.float32(half)
    ).astype(np.float32)
    a = t.astype(np.float32)[:, None] * freqs[None, :]
    return np.concatenate([np.sin(a), np.cos(a)], axis=-1).astype(np.float32)


def _host_tails(angle, mask_pos, time_pos, ang_w1, ang_w2, t_w1, t_b1, t_w2, t_b2):
    """rest[b, n, :] with rest[..., :INTER] = time_emb[..., :INTER] and
    rest[..., INTER:] = ang_f + time_emb[..., INTER:]."""
    angle = np.asarray(angle, np.float32)
    ang = np.where(np.isposinf(angle), np.float32(0.0), angle).astype(np.float32)
    ang_f = _gelu_np(ang @ np.asarray(ang_w1, np.float32)) @ np.asarray(
        ang_w2, np.float32
    )  # [B, N, INTER]

    def time_mlp(t):
        e = _timestep_emb_np(t, E)
        h = _silu_np(e @ np.asarray(t_w1, np.float32) + np.asarray(t_b1, np.float32))
        return (h @ np.asarray(t_w2, np.float32) + np.asarray(t_b2, np.float32)).astype(
            np.float32
        )

    tp = np.asarray(time_pos)
    te = time_mlp(tp)[:, None, :]                 # [B, 1, E]
    t0e = time_mlp(np.zeros_like(tp))[:, None, :]
    mask = np.asarray(mask_pos, bool)             # [B, N, 1]
    time_emb = np.where(mask, te, t0e).astype(np.float32)  # [B, N, E]

    rest = time_emb.copy()
    rest[..., INTER:] += ang_f.astype(np.float32)
    return rest.astype(np.float32)


def _prep_in_maps(pos, angle, padding_mask, mask_pos, time_pos,
                  means, stds, fp_w1, fp_w2, ang_w1, ang_w2,
                  t_w1, t_b1, t_w2, t_b2, use_fallback=None):
    if use_fallback is None:
        use_fallback = USE_FALLBACK_EXP
    pos = np.asarray(pos, np.float32)
    pad = np.asarray(padding_mask, bool)

    s = (np.abs(np.asarray(stds, np.float32)) + np.float32(0.01)).astype(np.float32)
    m = np.asarray(means, np.float32)
    inv_s = (np.float32(1.0) / s).astype(np.float32)
    if not use_fallback:
        # Derivative_Erf(x) with x = (d - m)/(s*sqrt(2))
        esc_v = (inv_s / np.float32(math.sqrt(2.0))).astype(np.float32)
        ebi_v = (-m * esc_v).astype(np.float32)
        postc_v = (
            np.float32(DERF_INV) / (np.float32(math.sqrt(2.0 * PI_REF)) * s)
        ).astype(np.float32)
    else:
        # Square then Exp(-0.5 z^2 + log c)
        esc_v = inv_s.astype(np.float32)
        ebi_v = (-m * inv_s).astype(np.float32)
        postc_v = np.log(
            np.float32(1.0) / (np.float32(math.sqrt(2.0 * PI_REF)) * s)
        ).astype(np.float32)

    blk3 = np.zeros((96, GR), np.float32)
    for p in range(96):
        blk3[p, p // 3] = 1.0

    rest = _host_tails(
        angle, mask_pos, time_pos, ang_w1, ang_w2, t_w1, t_b1, t_w2, t_b2
    )

    ident = np.eye(128, dtype=np.float32)
    w1_v = np.asarray(fp_w1, np.float32)
    w2_v = np.asarray(fp_w2, np.float32)

    in_maps = []
    for c in range(NCORES):
        b = c // (NCORES // B)
        r0 = (c % (NCORES // B)) * RPC
        posT = pos[b].T.copy()  # [3, N]
        if pad[b].any():
            posT[:, pad[b]] = np.float32(1.0e6)
        # phase-A partition rr holds the query row that lands on PE quadrant
        # 0 (first 8 of each 16-row subgroup) or quadrant 2 (last 8), so the
        # M_flat flatten DMAs stay partition-contiguous while consecutive
        # device rows alternate quadrants (rows a0,a0+1 -> Q0; a0+2,a0+3 -> Q2)
        perm16 = np.array([0, 1, 4, 5, 8, 9, 12, 13, 2, 3, 6, 7, 10, 11, 14, 15])
        perm = np.concatenate([perm16, 16 + perm16])
        qscal = np.empty((96, NGROUPS), np.float32)
        for g in range(NGROUPS):
            rows = pos[b, r0 + g * GR : r0 + (g + 1) * GR, :][perm]  # [32, 3]
            qscal[:, g] = rows.reshape(-1)
        in_maps.append(
            {
                "posT": np.ascontiguousarray(posT, np.float32),
                "qscal": qscal,
                "blk3": blk3,
                "esc": esc_v.reshape(K, 1),
                "ebi": ebi_v.reshape(K, 1),
                "postc": postc_v.reshape(K, 1),
                "w1": w1_v,
                "w2": w2_v,
                "ident": ident,
                "rest": np.ascontiguousarray(rest[b, r0 : r0 + RPC, :], np.float32),
            }
        )
    return in_maps


def kernel(pos, angle, node_type_edge, padding_mask, mask_aa, mask_pos, time_pos,
           means, stds, fp_w1, fp_w2, ang_w1, ang_w2, t_w1, t_b1, t_w2, t_b2):
    from concourse.bass_utils import run_bass_kernel_spmd

    key = ("nc", USE_FALLBACK_EXP, _FUNC_OVERRIDE)
    if key not in _COMPILED:
        _COMPILED[key] = _build_nc(func_override=_FUNC_OVERRIDE)
    nc = _COMPILED[key]

    in_maps = _prep_in_maps(
        pos, angle, padding_mask, mask_pos, time_pos, means, stds,
        fp_w1, fp_w2, ang_w1, ang_w2, t_w1, t_b1, t_w2, t_b2,
    )
    res = run_bass_kernel_spmd(nc, in_maps, core_ids=list(range(NCORES)))
    outs = [np.asarray(res.results[c]["out"], np.float32) for c in range(NCORES)]
    full = np.concatenate(outs, axis=0).reshape(B, N, E)
    return full



# revision 5
# speedup vs baseline: 1.7996x; 1.7996x over previous
"""Trainium2 Bass kernel for nn_Node3DEmbeddingv2 (gnn_message_passing).

Strategy (8 NeuronCores, SPMD, row-sharded: 4 cores per batch x 192 rows):

The reference needs sum_pf[i,k] = sum_j c_k * exp(-((d_ij - m_k)/s_k)^2 / 2)
for K=128 gaussian channels over N=768 keys. Evaluating all 128 channels per
pair is ACT-bound (1 elem/cycle/partition, dtype-independent). Instead:

  - Wide channels (s_k large) are representable as linear combinations of a
    small fixed set of gaussian BASIS functions of d (grid centers mu_t, width
    sig_t, evaluated by the same Derivative_Erf table). The key-axis sum
    commutes with the linear combination, so the device only evaluates
    T ~ 24 basis features + ~24 narrow channels exactly => F ~ 48 "features"
    instead of 128 channels (host least-squares fit, residual ~1e-3, final
    error ~3e-5 << 2e-2 tolerance).
  - The feature->channel projection P is folded into fp_w1 on host:
    node3d = gelu(S @ (P @ fp_w1)) @ fp_w2, so sum_pf is never materialized.
  - Distances via one augmented PE matmul (gram trick): rows [px,py,pz,r2,1]
    x keys [-2px,-2py,-2pz,1,r2] -> d^2 in PSUM; ACT sqrt -> d.
    Self-pairs (cancellation-unsafe) are excluded by accumulating 1e9 onto
    the diagonal (per-core selector matrix, one extra matmul) and the exact
    self-term q0_k = c_k exp(-m_k^2/2s_k^2) is folded into the gelu bias
    (q0 @ fp_w1) on host.
  - 192 rows/core = one full [128,768] tile + a 64-row remainder; the
    remainder is duplicated into both partition halves and evaluates TWO
    features per ACT pass using per-partition [128,1] scale/bias vectors,
    so ACT packing is perfect: 1.5*F instructions of 768 elems each.
  - Key-axis sums on DVE (reduce over batched [128,8,768] f16 tiles),
    overlapping the ACT stream; a few trailing features use the ACT
    accumulator instead to balance the two engines.
  - Host (numpy, negligible): basis fit, angle MLP, time-embedding MLP,
    per-core input prep; all heavy compute is on-device.
"""

import math

import numpy as np

# Problem constants (hardcoded per the task contract).
B, N, K, E = 2, 768, 128, 512
INTER = E // 2
NCORES = 8
RPC = (B * N) // NCORES  # 192 rows per core
PI_REF = 3.14159         # matches reference's gaussian constant

# Derivative_Erf table: d/dx erf(x) = 2/sqrt(pi) * exp(-x^2).
DERF_INV = math.sqrt(math.pi) / 2.0

# Basis-fit hyperparameters (host-side, cheap).
FIT_D0 = 0.7        # base grid spacing
FIT_GROWTH = 0.12   # spacing growth beyond d=3
FIT_SIGR = 1.1      # basis width / local spacing
FIT_TOL = 1e-3      # max abs residual (vs unit peak) to accept a channel
FIT_RIDGE = 1e-7
DIAG_BIG = 1.0e9    # added to d^2 on the diagonal to exclude self-pairs
PAD_BIG = 1.0e12    # added to key r2 for padded keys

G_RED = 8           # DVE reduce batch (features per reduce instruction)
N_ACCUM = 4         # trailing stream instrs routed to the ACT accumulator

_COMPILED = {}


def _build_nc(F):
    import concourse.bass as bass
    import concourse.bacc as bacc
    from concourse import mybir
    from concourse.tile import TileContext

    f32 = mybir.dt.float32
    f16 = mybir.dt.float16
    AF = mybir.ActivationFunctionType
    F2 = F // 2

    nc = bacc.Bacc("TRN2", target_bir_lowering=False)

    # DRAM I/O (per-core values supplied via in_maps).
    posk = nc.dram_tensor("posk", [5, N], f32, kind="ExternalInput")
    posq1 = nc.dram_tensor("posq1", [5, 128], f32, kind="ExternalInput")
    posq2 = nc.dram_tensor("posq2", [5, 128], f32, kind="ExternalInput")
    sel1 = nc.dram_tensor("sel1", [128, N], f32, kind="ExternalInput")
    sel2 = nc.dram_tensor("sel2", [128, N], f32, kind="ExternalInput")
    sm1 = nc.dram_tensor("sm1", [128, F], f32, kind="ExternalInput")
    bm1 = nc.dram_tensor("bm1", [128, F], f32, kind="ExternalInput")
    sm2 = nc.dram_tensor("sm2", [128, F2], f32, kind="ExternalInput")
    bm2 = nc.dram_tensor("bm2", [128, F2], f32, kind="ExternalInput")
    w1x = nc.dram_tensor("w1x", [F, 128], f32, kind="ExternalInput")
    w1xa = nc.dram_tensor("w1xa", [F2, 128], f32, kind="ExternalInput")
    w1xb = nc.dram_tensor("w1xb", [F2, 128], f32, kind="ExternalInput")
    qb = nc.dram_tensor("qb", [128, 1], f32, kind="ExternalInput")
    w2 = nc.dram_tensor("w2", [K, INTER], f32, kind="ExternalInput")
    ident = nc.dram_tensor("ident", [128, 128], f32, kind="ExternalInput")
    rest = nc.dram_tensor("rest", [RPC, E], f32, kind="ExternalInput")
    out = nc.dram_tensor("out", [RPC, E], f32, kind="ExternalOutput")

    with TileContext(nc) as tc:
        with tc.tile_pool(name="sb", bufs=1) as sb:
            # ---- constant loads ----
            posk_sb = sb.tile([5, N], f32, tag="posk")
            nc.sync.dma_start(out=posk_sb, in_=posk[:, :])
            posq1_sb = sb.tile([5, 128], f32, tag="posq1")
            nc.sync.dma_start(out=posq1_sb, in_=posq1[:, :])
            posq2_sb = sb.tile([5, 128], f32, tag="posq2")
            nc.sync.dma_start(out=posq2_sb, in_=posq2[:, :])
            sel1_sb = sb.tile([128, N], f32, tag="sel1")
            nc.sync.dma_start(out=sel1_sb, in_=sel1[:, :])
            sel2_sb = sb.tile([128, N], f32, tag="sel2")
            nc.sync.dma_start(out=sel2_sb, in_=sel2[:, :])
            sm1_sb = sb.tile([128, F], f32, tag="sm1")
            nc.sync.dma_start(out=sm1_sb, in_=sm1[:, :])
            bm1_sb = sb.tile([128, F], f32, tag="bm1")
            nc.sync.dma_start(out=bm1_sb, in_=bm1[:, :])
            sm2_sb = sb.tile([128, F2], f32, tag="sm2")
            nc.sync.dma_start(out=sm2_sb, in_=sm2[:, :])
            bm2_sb = sb.tile([128, F2], f32, tag="bm2")
            nc.sync.dma_start(out=bm2_sb, in_=bm2[:, :])
            w1x_sb = sb.tile([F, 128], f32, tag="w1x")
            nc.sync.dma_start(out=w1x_sb, in_=w1x[:, :])
            w1xa_sb = sb.tile([F2, 128], f32, tag="w1xa")
            nc.sync.dma_start(out=w1xa_sb, in_=w1xa[:, :])
            w1xb_sb = sb.tile([F2, 128], f32, tag="w1xb")
            nc.sync.dma_start(out=w1xb_sb, in_=w1xb[:, :])
            qb_sb = sb.tile([128, 1], f32, tag="qb")
            nc.sync.dma_start(out=qb_sb, in_=qb[:, :])
            w2_sb = sb.tile([K, INTER], f32, tag="w2")
            nc.sync.dma_start(out=w2_sb, in_=w2[:, :])
            id_sb = sb.tile([128, 128], f32, tag="ident")
            nc.sync.dma_start(out=id_sb, in_=ident[:, :])

            S1 = sb.tile([128, F], f32, tag="S1")
            S2 = sb.tile([128, F2], f32, tag="S2")

            # Collapse input-DMA queue semaphores into one point.
            tc.strict_bb_all_engine_barrier()

            # ---- phase A: d^2 via gram matmul, diag exclusion, sqrt ----
            with tc.tile_pool(name="psD", bufs=1, space="PSUM") as psD:
                d1 = psD.tile([128, N], f32, tag="d1")
                d2 = psD.tile([128, N], f32, tag="d2")
                with tc.tile_pool(name="psA", bufs=1, space="PSUM") as psA:
                    for dq, posq_sb, sel_sb in (
                        (d1, posq1_sb, sel1_sb),
                        (d2, posq2_sb, sel2_sb),
                    ):
                        dsq = psA.tile([128, N], f32, tag="dsq", bufs=2)
                        for lo, hi in ((0, 512), (512, N)):
                            nc.tensor.matmul(
                                dsq[:, lo:hi], posq_sb, posk_sb[:, lo:hi],
                                start=True, stop=False,
                            )
                            nc.tensor.matmul(
                                dsq[:, lo:hi], id_sb, sel_sb[:, lo:hi],
                                start=False, stop=True,
                            )
                        nc.scalar.sqrt(dq, dsq)

                # ---- phase B: feature evals (ACT) + key sums (DVE/accum) ----
                # stream entries: (d_tile, sm_tile, bm_tile, S_tile, col)
                stream = [(d1, sm1_sb, bm1_sb, S1, f) for f in range(F)] + [
                    (d2, sm2_sb, bm2_sb, S2, p) for p in range(F2)
                ]
                n_dve = len(stream) - N_ACCUM
                i = 0
                while i < n_dve:
                    gn = min(G_RED, n_dve - i)
                    gsc = sb.tile([128, G_RED, N], f16, tag="gsc", bufs=3)
                    for j in range(gn):
                        dt_, smt, bmt, _, col = stream[i + j]
                        nc.scalar.activation(
                            out=gsc[:, j, :], in_=dt_,
                            func=AF.Derivative_Erf,
                            bias=bmt[:, col : col + 1],
                            scale=smt[:, col : col + 1],
                        )
                    # reduce over the key axis into the S columns; the S
                    # columns within a group are contiguous by construction
                    # except across the S1/S2 boundary -> split there.
                    j0 = 0
                    while j0 < gn:
                        run = 1
                        while (
                            j0 + run < gn
                            and stream[i + j0 + run][3] is stream[i + j0][3]
                        ):
                            run += 1
                        St = stream[i + j0][3]
                        c0 = stream[i + j0][4]
                        nc.vector.reduce_sum(
                            out=St[:, c0 : c0 + run],
                            in_=gsc[:, j0 : j0 + run, :],
                            axis=mybir.AxisListType.X,
                        )
                        j0 += run
                    i += gn
                # trailing features -> ACT accumulator (balances DVE)
                for dt_, smt, bmt, St, col in stream[n_dve:]:
                    trash = sb.tile([128, 1, N], f16, tag="trash", bufs=2)
                    nc.scalar.activation(
                        out=trash[:, 0, :], in_=dt_,
                        func=AF.Derivative_Erf,
                        bias=bmt[:, col : col + 1],
                        scale=smt[:, col : col + 1],
                        accum_out=St[:, col : col + 1],
                    )

            # ---- phase C: transpose S, folded MLP, output ----
            with tc.tile_pool(name="psC", bufs=1, space="PSUM") as psC:
                pst1 = psC.tile([F, 128], f32, tag="pst1")
                nc.tensor.transpose(pst1, S1, id_sb)
                st1 = sb.tile([F, 128], f32, tag="st1")
                nc.vector.tensor_copy(st1, pst1)
                pst2 = psC.tile([F2, 128], f32, tag="pst2")
                nc.tensor.transpose(pst2, S2, id_sb)
                st2 = sb.tile([F2, 128], f32, tag="st2")
                nc.vector.tensor_copy(st2, pst2)

                psum_h = psC.tile([128, RPC], f32, tag="h")
                nc.tensor.matmul(
                    psum_h[:, 0:128], w1x_sb, st1, start=True, stop=True
                )
                nc.tensor.matmul(
                    psum_h[:, 128:RPC], w1xa_sb, st2[:, 0:64],
                    start=True, stop=False,
                )
                nc.tensor.matmul(
                    psum_h[:, 128:RPC], w1xb_sb, st2[:, 64:128],
                    start=False, stop=True,
                )
                h_sb = sb.tile([128, RPC], f32, tag="h_sb")
                nc.scalar.activation(h_sb, psum_h, AF.Gelu, bias=qb_sb)

                o_sb = sb.tile([128, 2, RPC], f32, tag="o_sb")
                for e in range(2):
                    psum_o = psC.tile([128, RPC], f32, tag="po", bufs=2)
                    nc.tensor.matmul(
                        psum_o, w2_sb[:, 128 * e : 128 * (e + 1)], h_sb,
                        start=True, stop=True,
                    )
                    nc.vector.tensor_copy(o_sb[:, e, :], psum_o)

                for t in range(2):
                    out_sb = sb.tile([96, E], f32, tag="outsb", bufs=2)
                    nc.gpsimd.dma_start(
                        out=out_sb, in_=rest[96 * t : 96 * (t + 1), :]
                    )
                    for e in range(2):
                        psum_t = psC.tile([96, 128], f32, tag="ptr", bufs=2)
                        nc.tensor.transpose(
                            psum_t, o_sb[:, e, 96 * t : 96 * (t + 1)], id_sb
                        )
                        nc.vector.tensor_add(
                            out_sb[:, 128 * e : 128 * (e + 1)],
                            out_sb[:, 128 * e : 128 * (e + 1)],
                            psum_t,
                        )
                    nc.sync.dma_start(
                        out=out[96 * t : 96 * (t + 1), :], in_=out_sb
                    )

    nc.compile()
    return nc


# ---------------- host-side computation (numpy, f32/f64) ----------------

def _erf_np(x):
    try:
        from scipy.special import erf
        return erf(x)
    except ImportError:
        f = np.frompyfunc(math.erf, 1, 1)
        return f(x.astype(np.float64)).astype(np.float64)


def _gelu_np(x):
    x = x.astype(np.float32)
    return (x * 0.5 * (1.0 + _erf_np(x / np.float32(math.sqrt(2.0))))).astype(
        np.float32
    )


def _silu_np(x):
    x = x.astype(np.float32)
    return (x / (1.0 + np.exp(-x))).astype(np.float32)


def _timestep_emb_np(t, dim):
    half = dim // 2
    freqs = np.exp(
        -np.log(10000.0) * np.arange(half, dtype=np.float32) / np.float32(half)
    ).astype(np.float32)
    a = t.astype(np.float32)[:, None] * freqs[None, :]
    return np.concatenate([np.sin(a), np.cos(a)], axis=-1).astype(np.float32)


def _host_tails(angle, mask_pos, time_pos, ang_w1, ang_w2, t_w1, t_b1, t_w2, t_b2):
    """rest[b, n, :] with rest[..., :INTER] = time_emb[..., :INTER] and
    rest[..., INTER:] = ang_f + time_emb[..., INTER:]."""
    angle = np.asarray(angle, np.float32)
    ang = np.where(np.isposinf(angle), np.float32(0.0), angle).astype(np.float32)
    ang_f = _gelu_np(ang @ np.asarray(ang_w1, np.float32)) @ np.asarray(
        ang_w2, np.float32
    )  # [B, N, INTER]

    def time_mlp(t):
        e = _timestep_emb_np(t, E)
        h = _silu_np(e @ np.asarray(t_w1, np.float32) + np.asarray(t_b1, np.float32))
        return (h @ np.asarray(t_w2, np.float32) + np.asarray(t_b2, np.float32)).astype(
            np.float32
        )

    tp = np.asarray(time_pos)
    te = time_mlp(tp)[:, None, :]                 # [B, 1, E]
    t0e = time_mlp(np.zeros_like(tp))[:, None, :]
    mask = np.asarray(mask_pos, bool)             # [B, N, 1]
    time_emb = np.where(mask, te, t0e).astype(np.float32)  # [B, N, E]

    rest = time_emb.copy()
    rest[..., INTER:] += ang_f.astype(np.float32)
    return rest.astype(np.float32)


def _derf_val(x):
    return 2.0 / math.sqrt(math.pi) * np.exp(-x * x)


def _make_grid(d0, growth, start=-1.0, dmax=18.6, sigr=1.1):
    mu = [start]
    while mu[-1] < dmax:
        step = max(d0, (mu[-1] - 3.0) * growth) if growth > 0 else d0
        mu.append(mu[-1] + step)
    mu = np.array(mu)
    steps = np.diff(mu)
    steps = np.append(steps, steps[-1])
    sig = np.maximum(d0, steps) * sigr
    return mu, sig


def _fit_basis(means, stds):
    """Fit the K gaussian channels on a grid basis + exact tail.

    Returns scales[F], biases[F], P[F, K] (f64) such that
      sum_pf[:, k] ~= sum_j derf(scales*d_j + biases) @ P[:, k]
    for off-diagonal pairs, where derf(x) = 2/sqrt(pi) exp(-x^2).
    """
    means = np.asarray(means, np.float64)
    s = np.abs(np.asarray(stds, np.float64)) + 0.01
    ck = 1.0 / (np.sqrt(2.0 * PI_REF) * s)

    mu, sig = _make_grid(FIT_D0, FIT_GROWTH, sigr=FIT_SIGR)
    T = len(mu)
    dg = np.linspace(0.0, 24.0, 4801)
    A = _derf_val((dg[:, None] - mu[None, :]) / (sig[None, :] * math.sqrt(2.0)))
    Gt = np.exp(-0.5 * ((dg[:, None] - means[None, :]) / s[None, :]) ** 2)
    AtA = A.T @ A + FIT_RIDGE * np.eye(T)
    coef = np.linalg.solve(AtA, A.T @ Gt)          # [T, K]
    resid = np.abs(A @ coef - Gt).max(axis=0)      # [K]
    grid_ch = resid < FIT_TOL
    exact = np.where(~grid_ch)[0]

    Fn = T + len(exact)
    if Fn > 128:
        # degenerate inputs: fall back to exact-only evaluation
        grid_ch = np.zeros(K, bool)
        exact = np.arange(K)
        mu = np.zeros((0,))
        sig = np.ones((0,))
        T = 0
        Fn = K

    scales = np.concatenate(
        [1.0 / (sig * math.sqrt(2.0)), 1.0 / (s[exact] * math.sqrt(2.0))]
    )
    biases = np.concatenate(
        [-mu / (sig * math.sqrt(2.0)), -means[exact] / (s[exact] * math.sqrt(2.0))]
    )
    P = np.zeros((Fn, K))
    if T:
        P[:T, grid_ch] = coef[:, grid_ch] * ck[grid_ch][None, :]
    for idx, k in enumerate(exact):
        P[T + idx, k] = DERF_INV * ck[k]

    if Fn % 2:  # pad to even for the 2-features-per-pass remainder trick
        scales = np.append(scales, 1.0)
        biases = np.append(biases, 1.0e4)  # derf(d + 1e4) == 0
        P = np.vstack([P, np.zeros((1, K))])
        Fn += 1

    # exact self-term (diagonal excluded on device): q0_k = ck*exp(-m^2/2s^2)
    q0 = ck * np.exp(-0.5 * (means / s) ** 2)
    return scales, biases, P, q0, Fn


def _prep_in_maps(pos, angle, padding_mask, mask_pos, time_pos,
                  means, stds, fp_w1, fp_w2, ang_w1, ang_w2,
                  t_w1, t_b1, t_w2, t_b2):
    pos = np.asarray(pos, np.float32)
    pad = np.asarray(padding_mask, bool)

    scales, biases, P, q0, F = _fit_basis(means, stds)
    F2 = F // 2
    w1x_v = (P @ np.asarray(fp_w1, np.float64)).astype(np.float32)   # [F, 128]
    qb_v = (q0 @ np.asarray(fp_w1, np.float64)).astype(np.float32)   # [128]
    scales32 = scales.astype(np.float32)
    biases32 = biases.astype(np.float32)

    # tile1: all 128 partitions share the feature's scale/bias
    sm1_v = np.repeat(scales32[None, :], 128, axis=0)
    bm1_v = np.repeat(biases32[None, :], 128, axis=0)
    # tile2 pairing (p, p+F2): partitions 0:64 -> feature p, 64:128 -> p+F2
    sm2_v = np.empty((128, F2), np.float32)
    bm2_v = np.empty((128, F2), np.float32)
    sm2_v[0:64, :] = scales32[None, :F2]
    sm2_v[64:128, :] = scales32[None, F2:F]
    bm2_v[0:64, :] = biases32[None, :F2]
    bm2_v[64:128, :] = biases32[None, F2:F]

    rest = _host_tails(
        angle, mask_pos, time_pos, ang_w1, ang_w2, t_w1, t_b1, t_w2, t_b2
    )

    ident = np.eye(128, dtype=np.float32)
    w2_v = np.asarray(fp_w2, np.float32)

    in_maps = []
    for c in range(NCORES):
        b = c // (NCORES // B)
        r0 = (c % (NCORES // B)) * RPC
        p = pos[b]                       # [N, 3]
        r2 = (p * p).sum(axis=1).astype(np.float32)          # [N]
        posk_v = np.empty((5, N), np.float32)
        posk_v[0:3] = (-2.0 * p.T).astype(np.float32)
        posk_v[3] = 1.0
        posk_v[4] = r2
        if pad[b].any():
            posk_v[4, pad[b]] += np.float32(PAD_BIG)

        def make_posq(rows):
            pq = np.empty((5, len(rows)), np.float32)
            pr = p[rows]
            pq[0:3] = pr.T
            pq[3] = r2[rows]
            pq[4] = 1.0
            return pq

        rows1 = np.arange(r0, r0 + 128)
        rows2d = np.concatenate(
            [np.arange(r0 + 128, r0 + 192), np.arange(r0 + 128, r0 + 192)]
        )
        sel1_v = np.zeros((128, N), np.float32)
        sel1_v[np.arange(128), rows1] = np.float32(DIAG_BIG)
        sel2_v = np.zeros((128, N), np.float32)
        sel2_v[np.arange(128), rows2d] = np.float32(DIAG_BIG)

        in_maps.append(
            {
                "posk": posk_v,
                "posq1": make_posq(rows1),
                "posq2": make_posq(rows2d),
                "sel1": sel1_v,
                "sel2": sel2_v,
                "sm1": sm1_v,
                "bm1": bm1_v,
                "sm2": sm2_v,
                "bm2": bm2_v,
                "w1x": w1x_v,
                "w1xa": np.ascontiguousarray(w1x_v[:F2]),
                "w1xb": np.ascontiguousarray(w1x_v[F2:]),
                "qb": qb_v.reshape(128, 1),
                "w2": w2_v,
                "ident": ident,
                "rest": np.ascontiguousarray(rest[b, r0 : r0 + RPC, :], np.float32),
            }
        )
    return in_maps, F


def kernel(pos, angle, node_type_edge, padding_mask, mask_aa, mask_pos, time_pos,
           means, stds, fp_w1, fp_w2, ang_w1, ang_w2, t_w1, t_b1, t_w2, t_b2):
    from concourse.bass_utils import run_bass_kernel_spmd

    in_maps, F = _prep_in_maps(
        pos, angle, padding_mask, mask_pos, time_pos, means, stds,
        fp_w1, fp_w2, ang_w1, ang_w2, t_w1, t_b1, t_w2, t_b2,
    )
    if F not in _COMPILED:
        _COMPILED[F] = _build_nc(F)
    nc = _COMPILED[F]
    res = run_bass_kernel_spmd(nc, in_maps, core_ids=list(range(NCORES)))
    outs = [np.asarray(res.results[c]["out"], np.float32) for c in range(NCORES)]
    full = np.concatenate(outs, axis=0).reshape(B, N, E)
    return full


# revision 9
# speedup vs baseline: 2.0220x; 1.1236x over previous
"""Trainium2 Bass kernel for nn_Node3DEmbeddingv2 (gnn_message_passing).

Strategy (8 NeuronCores, SPMD, row-sharded: 4 cores per batch x 192 rows):

The reference needs sum_pf[i,k] = sum_j c_k * exp(-((d_ij - m_k)/s_k)^2 / 2)
for K=128 gaussian channels over N=768 keys. Evaluating all 128 channels per
pair is ACT-bound (1 elem/cycle/partition, dtype-independent). Instead:

  - Wide channels (s_k large) are representable as linear combinations of a
    small fixed set of gaussian BASIS functions of d (grid centers mu_t, width
    sig_t, evaluated by the same Derivative_Erf table). The key-axis sum
    commutes with the linear combination, so the device only evaluates
    T ~ 24 basis features + ~24 narrow channels exactly => F ~ 48 "features"
    instead of 128 channels (host least-squares fit, residual ~1e-3, final
    error ~3e-5 << 2e-2 tolerance).
  - The feature->channel projection P is folded into fp_w1 on host:
    node3d = gelu(S @ (P @ fp_w1)) @ fp_w2, so sum_pf is never materialized.
  - Distances via one augmented PE matmul (gram trick): rows [px,py,pz,r2,1]
    x keys [-2px,-2py,-2pz,1,r2] -> d^2 in PSUM (clamped at 0 on DVE against
    f32 cancellation on the diagonal); ACT sqrt -> d kept in PSUM.
  - 192 rows/core = one full [128,768] tile + a 64-row remainder; the
    remainder is duplicated into both partition halves and evaluates TWO
    features per ACT pass using per-partition [128,1] scale/bias vectors,
    so ACT packing is perfect: 1.5*F instructions of 768 elems each.
  - Key-axis sums via DVE tensor_tensor_reduce: one instr per feature adds
    the two key-halves (f16, 2x mode) and reduces into S[:, f] with f32
    accumulation — DVE stays far off the ACT critical path.
  - Only the phase-A/B inputs gate the start barrier; MLP weights stream in
    during the gaussian stream.
  - Host (numpy, negligible): basis fit, angle MLP, time-embedding MLP,
    per-core input prep; all heavy compute is on-device.
"""

import math

import numpy as np

# Problem constants (hardcoded per the task contract).
B, N, K, E = 2, 768, 128, 512
INTER = E // 2
NCORES = 8
RPC = (B * N) // NCORES  # 192 rows per core
PI_REF = 3.14159         # matches reference's gaussian constant

# Derivative_Erf table: d/dx erf(x) = 2/sqrt(pi) * exp(-x^2).
DERF_INV = math.sqrt(math.pi) / 2.0

# Basis-fit hyperparameters (host-side, cheap).
FIT_D0 = 0.7        # base grid spacing
FIT_GROWTH = 0.12   # spacing growth beyond d=3
FIT_SIGR = 1.1      # basis width / local spacing
FIT_TOL = 1e-3      # max abs residual (vs unit peak) to accept a channel
FIT_RIDGE = 1e-7
PAD_BIG = 1.0e12    # added to key r2 for padded keys

import os
REDUCE_MODE = os.environ.get("N3D_REDUCE", "ttr")   # 'ttr' | 'reduce'
CLAMP_MODE = os.environ.get("N3D_CLAMP", "inplace")  # 'inplace' | 'copy'

_COMPILED = {}


def _build_nc(F):
    import concourse.bass as bass
    import concourse.bacc as bacc
    from concourse import mybir
    from concourse.tile import TileContext

    f32 = mybir.dt.float32
    f16 = mybir.dt.float16
    AF = mybir.ActivationFunctionType
    ALU = mybir.AluOpType
    F2 = F // 2

    nc = bacc.Bacc("TRN2", target_bir_lowering=False)

    # DRAM I/O (per-core values supplied via in_maps).
    posk = nc.dram_tensor("posk", [5, N], f32, kind="ExternalInput")
    posq1 = nc.dram_tensor("posq1", [5, 128], f32, kind="ExternalInput")
    posq2 = nc.dram_tensor("posq2", [5, 128], f32, kind="ExternalInput")
    sm1 = nc.dram_tensor("sm1", [128, F], f32, kind="ExternalInput")
    bm1 = nc.dram_tensor("bm1", [128, F], f32, kind="ExternalInput")
    sm2 = nc.dram_tensor("sm2", [128, F2], f32, kind="ExternalInput")
    bm2 = nc.dram_tensor("bm2", [128, F2], f32, kind="ExternalInput")
    w1x = nc.dram_tensor("w1x", [F, 128], f32, kind="ExternalInput")
    w1xa = nc.dram_tensor("w1xa", [F2, 128], f32, kind="ExternalInput")
    w1xb = nc.dram_tensor("w1xb", [F2, 128], f32, kind="ExternalInput")
    w2 = nc.dram_tensor("w2", [K, INTER], f32, kind="ExternalInput")
    ident = nc.dram_tensor("ident", [128, 128], f32, kind="ExternalInput")
    rest = nc.dram_tensor("rest", [RPC, E], f32, kind="ExternalInput")
    out = nc.dram_tensor("out", [RPC, E], f32, kind="ExternalOutput")

    with TileContext(nc) as tc:
        with tc.tile_pool(name="sb", bufs=1) as sb:
            # ---- critical-path loads (phase A/B inputs only) ----
            posk_sb = sb.tile([5, N], f32, tag="posk")
            nc.sync.dma_start(out=posk_sb, in_=posk[:, :])
            posq1_sb = sb.tile([5, 128], f32, tag="posq1")
            nc.sync.dma_start(out=posq1_sb, in_=posq1[:, :])
            posq2_sb = sb.tile([5, 128], f32, tag="posq2")
            nc.sync.dma_start(out=posq2_sb, in_=posq2[:, :])
            sm1_sb = sb.tile([128, F], f32, tag="sm1")
            nc.gpsimd.dma_start(out=sm1_sb, in_=sm1[:, :])
            bm1_sb = sb.tile([128, F], f32, tag="bm1")
            nc.gpsimd.dma_start(out=bm1_sb, in_=bm1[:, :])
            sm2_sb = sb.tile([128, F2], f32, tag="sm2")
            nc.gpsimd.dma_start(out=sm2_sb, in_=sm2[:, :])
            bm2_sb = sb.tile([128, F2], f32, tag="bm2")
            nc.gpsimd.dma_start(out=bm2_sb, in_=bm2[:, :])

            S1 = sb.tile([128, F], f32, tag="S1")
            S2 = sb.tile([128, F2], f32, tag="S2")

            # Collapse the critical input-DMA semaphores into one point.
            tc.strict_bb_all_engine_barrier()

            # ---- late loads (phase C inputs), overlap the gaussian stream
            w1x_sb = sb.tile([F, 128], f32, tag="w1x")
            nc.sync.dma_start(out=w1x_sb, in_=w1x[:, :])
            w1xa_sb = sb.tile([F2, 128], f32, tag="w1xa")
            nc.sync.dma_start(out=w1xa_sb, in_=w1xa[:, :])
            w1xb_sb = sb.tile([F2, 128], f32, tag="w1xb")
            nc.sync.dma_start(out=w1xb_sb, in_=w1xb[:, :])
            w2_sb = sb.tile([K, INTER], f32, tag="w2")
            nc.sync.dma_start(out=w2_sb, in_=w2[:, :])
            id_sb = sb.tile([128, 128], f32, tag="ident")
            nc.sync.dma_start(out=id_sb, in_=ident[:, :])

            # ---- phase A: d^2 via gram matmul, clamp, sqrt ----
            with tc.tile_pool(name="psD", bufs=1, space="PSUM") as psD:
                d1 = psD.tile([128, N], f32, tag="d1")
                d2 = psD.tile([128, N], f32, tag="d2")
                with tc.tile_pool(name="psA", bufs=1, space="PSUM") as psA:
                    dsqs = []
                    for tag, posq_sb in (
                        ("dsq1", posq1_sb), ("dsq2", posq2_sb),
                    ):
                        dsq = psA.tile([128, N], f32, tag=tag)
                        for lo, hi in ((0, 512), (512, N)):
                            nc.tensor.matmul(
                                dsq[:, lo:hi], posq_sb, posk_sb[:, lo:hi],
                                start=True, stop=True,
                            )
                        # clamp f32 cancellation on the diagonal (d_ii ~ 0)
                        if CLAMP_MODE == "inplace":
                            nc.vector.tensor_scalar_max(dsq, dsq, 0.0)
                        else:
                            dcl = psA.tile([128, N], f32, tag=tag + "c")
                            nc.vector.tensor_scalar_max(dcl, dsq, 0.0)
                            dsq = dcl
                        dsqs.append(dsq)
                    nc.scalar.sqrt(d1, dsqs[0])
                    nc.scalar.sqrt(d2, dsqs[1])

                # ---- phase B: feature evals (ACT) + key sums (DVE ttr) ----
                stream = [(d1, sm1_sb, bm1_sb, S1, f) for f in range(F)] + [
                    (d2, sm2_sb, bm2_sb, S2, p) for p in range(F2)
                ]
                for dt_, smt, bmt, St, col in stream:
                    gsc = sb.tile([128, N], f16, tag="gsc", bufs=6)
                    nc.scalar.activation(
                        out=gsc, in_=dt_,
                        func=AF.Derivative_Erf,
                        bias=bmt[:, col : col + 1],
                        scale=smt[:, col : col + 1],
                    )
                    if REDUCE_MODE == "ttr":
                        tt = sb.tile([128, N // 2], f16, tag="tt", bufs=2)
                        nc.vector.tensor_tensor_reduce(
                            out=tt,
                            in0=gsc[:, 0 : N // 2],
                            in1=gsc[:, N // 2 : N],
                            scale=1.0,
                            scalar=0.0,
                            op0=ALU.add,
                            op1=ALU.add,
                            accum_out=St[:, col : col + 1],
                        )
                    elif REDUCE_MODE == "ttr_bcast":
                        tt = sb.tile([128, 1], f16, tag="ttb", bufs=2)
                        nc.vector.tensor_tensor_reduce(
                            out=tt.broadcast_to((128, N // 2)),
                            in0=gsc[:, 0 : N // 2],
                            in1=gsc[:, N // 2 : N],
                            scale=1.0,
                            scalar=0.0,
                            op0=ALU.add,
                            op1=ALU.add,
                            accum_out=St[:, col : col + 1],
                        )
                    elif REDUCE_MODE == "ttr_f32":
                        tt = sb.tile([128, N // 2], f32, tag="ttf", bufs=2)
                        nc.vector.tensor_tensor_reduce(
                            out=tt,
                            in0=gsc[:, 0 : N // 2],
                            in1=gsc[:, N // 2 : N],
                            scale=1.0,
                            scalar=0.0,
                            op0=ALU.add,
                            op1=ALU.add,
                            accum_out=St[:, col : col + 1],
                        )
                    else:
                        nc.vector.reduce_sum(
                            out=St[:, col : col + 1],
                            in_=gsc,
                            axis=mybir.AxisListType.X,
                        )

            # ---- phase C: transpose S, folded MLP, output ----
            with tc.tile_pool(name="psC", bufs=1, space="PSUM") as psC:
                pst1 = psC.tile([F, 128], f32, tag="pst1")
                nc.tensor.transpose(pst1, S1, id_sb)
                st1 = sb.tile([F, 128], f32, tag="st1")
                nc.vector.tensor_copy(st1, pst1)
                pst2 = psC.tile([F2, 128], f32, tag="pst2")
                nc.tensor.transpose(pst2, S2, id_sb)
                st2 = sb.tile([F2, 128], f32, tag="st2")
                nc.vector.tensor_copy(st2, pst2)

                psum_h = psC.tile([128, RPC], f32, tag="h")
                nc.tensor.matmul(
                    psum_h[:, 0:128], w1x_sb, st1, start=True, stop=True
                )
                nc.tensor.matmul(
                    psum_h[:, 128:RPC], w1xa_sb, st2[:, 0:64],
                    start=True, stop=False,
                )
                nc.tensor.matmul(
                    psum_h[:, 128:RPC], w1xb_sb, st2[:, 64:128],
                    start=False, stop=True,
                )
                h_sb = sb.tile([128, RPC], f32, tag="h_sb")
                nc.scalar.activation(h_sb, psum_h, AF.Gelu)

                o_sb = sb.tile([128, 2, RPC], f32, tag="o_sb")
                for e in range(2):
                    psum_o = psC.tile([128, RPC], f32, tag="po", bufs=2)
                    nc.tensor.matmul(
                        psum_o, w2_sb[:, 128 * e : 128 * (e + 1)], h_sb,
                        start=True, stop=True,
                    )
                    nc.vector.tensor_copy(o_sb[:, e, :], psum_o)

                for t in range(2):
                    out_sb = sb.tile([96, E], f32, tag="outsb", bufs=2)
                    nc.gpsimd.dma_start(
                        out=out_sb, in_=rest[96 * t : 96 * (t + 1), :]
                    )
                    for e in range(2):
                        psum_t = psC.tile([96, 128], f32, tag="ptr", bufs=2)
                        nc.tensor.transpose(
                            psum_t, o_sb[:, e, 96 * t : 96 * (t + 1)], id_sb
                        )
                        nc.vector.tensor_add(
                            out_sb[:, 128 * e : 128 * (e + 1)],
                            out_sb[:, 128 * e : 128 * (e + 1)],
                            psum_t,
                        )
                    nc.sync.dma_start(
                        out=out[96 * t : 96 * (t + 1), :], in_=out_sb
                    )

    nc.compile()
    return nc


# ---------------- host-side computation (numpy, f32/f64) ----------------

def _erf_np(x):
    try:
        from scipy.special import erf
        return erf(x)
    except ImportError:
        f = np.frompyfunc(math.erf, 1, 1)
        return f(x.astype(np.float64)).astype(np.float64)


def _gelu_np(x):
    x = x.astype(np.float32)
    return (x * 0.5 * (1.0 + _erf_np(x / np.float32(math.sqrt(2.0))))).astype(
        np.float32
    )


def _silu_np(x):
    x = x.astype(np.float32)
    return (x / (1.0 + np.exp(-x))).astype(np.float32)


def _timestep_emb_np(t, dim):
    half = dim // 2
    freqs = np.exp(
        -np.log(10000.0) * np.arange(half, dtype=np.float32) / np.float32(half)
    ).astype(np.float32)
    a = t.astype(np.float32)[:, None] * freqs[None, :]
    return np.concatenate([np.sin(a), np.cos(a)], axis=-1).astype(np.float32)


def _host_tails(angle, mask_pos, time_pos, ang_w1, ang_w2, t_w1, t_b1, t_w2, t_b2):
    """rest[b, n, :] with rest[..., :INTER] = time_emb[..., :INTER] and
    rest[..., INTER:] = ang_f + time_emb[..., INTER:]."""
    angle = np.asarray(angle, np.float32)
    ang = np.where(np.isposinf(angle), np.float32(0.0), angle).astype(np.float32)
    ang_f = _gelu_np(ang @ np.asarray(ang_w1, np.float32)) @ np.asarray(
        ang_w2, np.float32
    )  # [B, N, INTER]

    def time_mlp(t):
        e = _timestep_emb_np(t, E)
        h = _silu_np(e @ np.asarray(t_w1, np.float32) + np.asarray(t_b1, np.float32))
        return (h @ np.asarray(t_w2, np.float32) + np.asarray(t_b2, np.float32)).astype(
            np.float32
        )

    tp = np.asarray(time_pos)
    te = time_mlp(tp)[:, None, :]                 # [B, 1, E]
    t0e = time_mlp(np.zeros_like(tp))[:, None, :]
    mask = np.asarray(mask_pos, bool)             # [B, N, 1]
    time_emb = np.where(mask, te, t0e).astype(np.float32)  # [B, N, E]

    rest = time_emb.copy()
    rest[..., INTER:] += ang_f.astype(np.float32)
    return rest.astype(np.float32)


def _derf_val(x):
    return 2.0 / math.sqrt(math.pi) * np.exp(-x * x)


def _make_grid(d0, growth, start=-1.0, dmax=18.6, sigr=1.1):
    mu = [start]
    while mu[-1] < dmax:
        step = max(d0, (mu[-1] - 3.0) * growth) if growth > 0 else d0
        mu.append(mu[-1] + step)
    mu = np.array(mu)
    steps = np.diff(mu)
    steps = np.append(steps, steps[-1])
    sig = np.maximum(d0, steps) * sigr
    return mu, sig


def _fit_basis(means, stds):
    """Fit the K gaussian channels on a grid basis + exact tail.

    Returns scales[F], biases[F], P[F, K] (f64) such that
      sum_pf[:, k] ~= sum_j derf(scales*d_j + biases) @ P[:, k]
    where derf(x) = 2/sqrt(pi) exp(-x^2).
    """
    means = np.asarray(means, np.float64)
    s = np.abs(np.asarray(stds, np.float64)) + 0.01
    ck = 1.0 / (np.sqrt(2.0 * PI_REF) * s)

    mu, sig = _make_grid(FIT_D0, FIT_GROWTH, sigr=FIT_SIGR)
    T = len(mu)
    dg = np.linspace(0.0, 24.0, 4801)
    A = _derf_val((dg[:, None] - mu[None, :]) / (sig[None, :] * math.sqrt(2.0)))
    Gt = np.exp(-0.5 * ((dg[:, None] - means[None, :]) / s[None, :]) ** 2)
    AtA = A.T @ A + FIT_RIDGE * np.eye(T)
    coef = np.linalg.solve(AtA, A.T @ Gt)          # [T, K]
    resid = np.abs(A @ coef - Gt).max(axis=0)      # [K]
    grid_ch = resid < FIT_TOL
    exact = np.where(~grid_ch)[0]

    Fn = T + len(exact)
    if Fn > 128:
        # degenerate inputs: fall back to exact-only evaluation
        grid_ch = np.zeros(K, bool)
        exact = np.arange(K)
        mu = np.zeros((0,))
        sig = np.ones((0,))
        T = 0
        Fn = K

    scales = np.concatenate(
        [1.0 / (sig * math.sqrt(2.0)), 1.0 / (s[exact] * math.sqrt(2.0))]
    )
    biases = np.concatenate(
        [-mu / (sig * math.sqrt(2.0)), -means[exact] / (s[exact] * math.sqrt(2.0))]
    )
    P = np.zeros((Fn, K))
    if T:
        P[:T, grid_ch] = coef[:, grid_ch] * ck[grid_ch][None, :]
    for idx, k in enumerate(exact):
        P[T + idx, k] = DERF_INV * ck[k]

    if Fn % 2:  # pad to even for the 2-features-per-pass remainder trick
        scales = np.append(scales, 1.0)
        biases = np.append(biases, 1.0e4)  # derf(d + 1e4) == 0
        P = np.vstack([P, np.zeros((1, K))])
        Fn += 1
    return scales, biases, P, Fn


def _prep_in_maps(pos, angle, padding_mask, mask_pos, time_pos,
                  means, stds, fp_w1, fp_w2, ang_w1, ang_w2,
                  t_w1, t_b1, t_w2, t_b2):
    pos = np.asarray(pos, np.float32)
    pad = np.asarray(padding_mask, bool)

    scales, biases, P, F = _fit_basis(means, stds)
    F2 = F // 2
    w1x_v = (P @ np.asarray(fp_w1, np.float64)).astype(np.float32)   # [F, 128]
    scales32 = scales.astype(np.float32)
    biases32 = biases.astype(np.float32)

    # tile1: all 128 partitions share the feature's scale/bias
    sm1_v = np.repeat(scales32[None, :], 128, axis=0)
    bm1_v = np.repeat(biases32[None, :], 128, axis=0)
    # tile2 pairing (p, p+F2): partitions 0:64 -> feature p, 64:128 -> p+F2
    sm2_v = np.empty((128, F2), np.float32)
    bm2_v = np.empty((128, F2), np.float32)
    sm2_v[0:64, :] = scales32[None, :F2]
    sm2_v[64:128, :] = scales32[None, F2:F]
    bm2_v[0:64, :] = biases32[None, :F2]
    bm2_v[64:128, :] = biases32[None, F2:F]

    rest = _host_tails(
        angle, mask_pos, time_pos, ang_w1, ang_w2, t_w1, t_b1, t_w2, t_b2
    )

    ident = np.eye(128, dtype=np.float32)
    w2_v = np.asarray(fp_w2, np.float32)

    in_maps = []
    for c in range(NCORES):
        b = c // (NCORES // B)
        r0 = (c % (NCORES // B)) * RPC
        p = pos[b]                       # [N, 3]
        r2 = (p * p).sum(axis=1).astype(np.float32)          # [N]
        posk_v = np.empty((5, N), np.float32)
        posk_v[0:3] = (-2.0 * p.T).astype(np.float32)
        posk_v[3] = 1.0
        posk_v[4] = r2
        if pad[b].any():
            posk_v[4, pad[b]] += np.float32(PAD_BIG)

        def make_posq(rows):
            pq = np.empty((5, len(rows)), np.float32)
            pr = p[rows]
            pq[0:3] = pr.T
            pq[3] = r2[rows]
            pq[4] = 1.0
            return pq

        rows1 = np.arange(r0, r0 + 128)
        rows2d = np.concatenate(
            [np.arange(r0 + 128, r0 + 192), np.arange(r0 + 128, r0 + 192)]
        )
        in_maps.append(
            {
                "posk": posk_v,
                "posq1": make_posq(rows1),
                "posq2": make_posq(rows2d),
                "sm1": sm1_v,
                "bm1": bm1_v,
                "sm2": sm2_v,
                "bm2": bm2_v,
                "w1x": w1x_v,
                "w1xa": np.ascontiguousarray(w1x_v[:F2]),
                "w1xb": np.ascontiguousarray(w1x_v[F2:]),
                "w2": w2_v,
                "ident": ident,
                "rest": np.ascontiguousarray(rest[b, r0 : r0 + RPC, :], np.float32),
            }
        )
    return in_maps, F


def kernel(pos, angle, node_type_edge, padding_mask, mask_aa, mask_pos, time_pos,
           means, stds, fp_w1, fp_w2, ang_w1, ang_w2, t_w1, t_b1, t_w2, t_b2):
    from concourse.bass_utils import run_bass_kernel_spmd

    in_maps, F = _prep_in_maps(
        pos, angle, padding_mask, mask_pos, time_pos, means, stds,
        fp_w1, fp_w2, ang_w1, ang_w2, t_w1, t_b1, t_w2, t_b2,
    )
    if F not in _COMPILED:
        _COMPILED[F] = _build_nc(F)
    nc = _COMPILED[F]
    res = run_bass_kernel_spmd(nc, in_maps, core_ids=list(range(NCORES)))
    outs = [np.asarray(res.results[c]["out"], np.float32) for c in range(NCORES)]
    full = np.concatenate(outs, axis=0).reshape(B, N, E)
    return full
